# revision 1
# baseline (speedup 1.0000x reference)
"""Trainium2 Bass kernel for a 2-layer cross-encoder (CrossEncoder).

Model: B=2, NQ=NKV=2048, E=512, H=8 (d_head=64), MLP=2048, depth=2, fp32 I/O.

Sharding (8 cores, no collectives): core c handles batch b=c//4 and query
rows [qc*512, (qc+1)*512) with qc=c%4.  Each core computes the full KV
projections for its batch (duplicated across the 4 cores sharing a batch)
so every core produces its output slice independently.

Numerics: matmul operands are bf16 (fp32 PSUM accumulation everywhere);
residual stream, LayerNorm statistics and softmax normalization stay fp32.
LN gamma/beta are folded into the projection weights on the host.  The
softmax denominator comes free from a ones-column appended to V (rows of
softmax sum to one, and the un-normalized attn@V matmul also computes
sum(exp) in the extra column).  exp() needs no max-subtraction: scores are
O(1) here (weights scaled 0.02), so exp is well-conditioned.
"""

import numpy as np
import ml_dtypes

import concourse.bass as bass
import concourse.bacc as bacc
import concourse.mybir as mybir
import concourse.tile as tile
from concourse import bass_utils, masks
from contextlib import ExitStack

P = 128
E = 512
EC = E // P        # 4 chunks of the embedding dim
NQ = 512           # query rows per core
QC = NQ // P       # 4 query chunks
NKV = 2048
KC = NKV // P      # 16 key chunks of 128
KN = NKV // 512    # 4 key chunks of 512
H = 8
DH = 64
MLP = 2048
MC = MLP // P      # 16 mlp chunks of 128
L = 2
LN_EPS = 1e-5
F32 = mybir.dt.float32
BF16 = mybir.dt.bfloat16
F32R = mybir.dt.float32r
AF = mybir.ActivationFunctionType
ALU = mybir.AluOpType
SCALE = (E // H) ** -0.5

_CACHE = {}


def _build():
    """Build the per-core Bass program (identical on all 8 cores)."""
    nc = bacc.Bacc("TRN2", target_bir_lowering=False, debug=False, num_devices=8)

    xq_d = nc.dram_tensor("xq", [NQ, E], F32, kind="ExternalInput").ap()
    xkv_d = nc.dram_tensor("xkv", [NKV, E], F32, kind="ExternalInput").ap()
    wd = []
    for l in range(L):
        wd.append({
            "wq": nc.dram_tensor(f"wq{l}", [P, EC * E], BF16, kind="ExternalInput").ap(),
            "wk": nc.dram_tensor(f"wk{l}", [P, EC * E], BF16, kind="ExternalInput").ap(),
            "wv": nc.dram_tensor(f"wv{l}", [P, EC * E], BF16, kind="ExternalInput").ap(),
            "wo": nc.dram_tensor(f"wo{l}", [P, EC * E], BF16, kind="ExternalInput").ap(),
            "w1": nc.dram_tensor(f"w1{l}", [P, EC * MLP], BF16, kind="ExternalInput").ap(),
            "w2": nc.dram_tensor(f"w2{l}", [P, MC * E], BF16, kind="ExternalInput").ap(),
            "bq": nc.dram_tensor(f"bq{l}", [P, EC], F32, kind="ExternalInput").ap(),
            "bk": nc.dram_tensor(f"bk{l}", [P, EC], F32, kind="ExternalInput").ap(),
            "b1": nc.dram_tensor(f"b1{l}", [P, MC], F32, kind="ExternalInput").ap(),
            "bo": nc.dram_tensor(f"bo{l}", [P, E], F32, kind="ExternalInput").ap(),
            "b2": nc.dram_tensor(f"b2{l}", [P, E], F32, kind="ExternalInput").ap(),
        })
    y_d = nc.dram_tensor("y", [NQ, E], F32, kind="ExternalOutput").ap()

    with tile.TileContext(nc) as tc, ExitStack() as ctx:
        const_pool = ctx.enter_context(tc.tile_pool(name="const", bufs=1))
        ident = const_pool.tile([P, P], BF16)
        masks.make_identity(nc, ident)
        ones_row = const_pool.tile([1, DH], BF16)
        nc.gpsimd.memset(ones_row[:], 1.0)
        eps_col = const_pool.tile([P, 1], F32)
        nc.gpsimd.memset(eps_col[:], LN_EPS)

        stats_pool = ctx.enter_context(tc.tile_pool(name="stats", bufs=12))

        def ln_tile(x_t, out_pool, out_name, apply_eng=None):
            """LayerNorm core (x - mu) * rsqrt(var + eps), fp32 in, bf16 out."""
            bnst = stats_pool.tile([P, 6], F32, name="bnst")
            nc.vector.bn_stats(bnst[:], x_t)
            bnag = stats_pool.tile([P, 2], F32, name="bnag")
            nc.vector.bn_aggr(bnag[:], bnst[:])
            sq = stats_pool.tile([P, 1], F32, name="sq")
            nc.scalar.activation(sq[:], bnag[:, 1:2], AF.Sqrt, bias=eps_col[:])
            rstd = stats_pool.tile([P, 1], F32, name="rstd")
            nc.vector.reciprocal(rstd[:], sq[:])
            h_t = out_pool.tile([P, E], BF16, name=out_name, bufs=4)
            (apply_eng or nc.gpsimd).tensor_scalar(
                h_t[:], x_t, bnag[:, 0:1], rstd[:], op0=ALU.subtract, op1=ALU.mult
            )
            return h_t

        # Residual stream: 4 fp32 tiles of [128, 512].
        xq_pool = ctx.enter_context(tc.tile_pool(name="xq", bufs=1))
        xq = []
        for i in range(QC):
            t = xq_pool.tile([P, E], F32, name=f"xq{i}", tag=f"xq{i}")
            nc.sync.dma_start(t[:], xq_d[i * P:(i + 1) * P, :])
            xq.append(t[:])
        xq_all = None

        # hkv^T: LN1-core of x_kv, transposed to [E, NKV].  ln1 g/b are folded
        # into the weights, so this is layer-independent: compute once.
        hkvT_pool = ctx.enter_context(tc.tile_pool(name="hkvT", bufs=1))
        hkvT = [
            hkvT_pool.tile([P, NKV], BF16, name=f"hkvT{e}", tag=f"hkvT{e}")
            for e in range(EC)
        ]

        # PSUM pools (8 banks total): pp 2 + ps_s 2x2 + ps_oT 2 = 8.
        pp_pool = ctx.enter_context(tc.tile_pool(name="pp", bufs=2, space="PSUM"))
        ss_pool = ctx.enter_context(tc.tile_pool(name="ss", bufs=2, space="PSUM"))
        att_pool = ctx.enter_context(tc.tile_pool(name="attp", bufs=2, space="PSUM"))

        def transpose_block(dst, src_block, copy_engine="vector"):
            """dst[128, 128] (slice of an SBUF tile) = src_block.T via PE."""
            pt = pp_pool.tile([P, E], F32, name="pp", tag="pp")
            ptb = pt[:].bitcast(BF16)[:, 0:P]
            nc.tensor.transpose(ptb, src_block, ident[:])
            if copy_engine == "scalar":
                nc.scalar.copy(dst, ptb)
            else:
                nc.vector.tensor_copy(dst, ptb)

        # Weight pools (bufs=2 -> next layer prefetches during current layer).
        wpool = ctx.enter_context(tc.tile_pool(name="w", bufs=2))

        def alloc_weights_crit(w):
            d = {}
            d["wq"] = wpool.tile([P, EC * E], BF16, name="wq_sb", tag="wq")
            nc.sync.dma_start(d["wq"][:], w["wq"])
            d["wk"] = wpool.tile([P, EC * E], BF16, name="wk_sb", tag="wk")
            nc.sync.dma_start(d["wk"][:], w["wk"])
            d["wv"] = wpool.tile([P, EC * E], BF16, name="wv_sb", tag="wv")
            nc.sync.dma_start(d["wv"][:], w["wv"])
            d["bq"] = wpool.tile([P, EC], F32, name="bq_sb", tag="bq")
            nc.sync.dma_start(d["bq"][:], w["bq"])
            d["bk"] = wpool.tile([P, EC], F32, name="bk_sb", tag="bk")
            nc.sync.dma_start(d["bk"][:], w["bk"])
            return d

        def alloc_weights_rest(d, w):
            d["wo"] = wpool.tile([P, EC * E], BF16, name="wo_sb", tag="wo")
            nc.sync.dma_start(d["wo"][:], w["wo"])
            d["b1"] = wpool.tile([P, MC], F32, name="b1_sb", tag="b1")
            nc.sync.dma_start(d["b1"][:], w["b1"])
            d["bo"] = wpool.tile([P, E], F32, name="bo_sb", tag="bo", bufs=1)
            nc.sync.dma_start(d["bo"][:], w["bo"])
            d["b2"] = wpool.tile([P, E], F32, name="b2_sb", tag="b2", bufs=1)
            nc.sync.dma_start(d["b2"][:], w["b2"])
            d["w1"] = wpool.tile([P, EC * MLP], BF16, name="w1_sb", tag="w1", bufs=1)
            nc.sync.dma_start(d["w1"][:], w["w1"])
            d["w2"] = wpool.tile([P, MC * E], BF16, name="w2_sb", tag="w2", bufs=1)
            nc.sync.dma_start(d["w2"][:], w["w2"])
            return d

        w0 = None
        with tc.tile_pool(name="xkv", bufs=4) as xkv_pool:
            for ib in range(KC // 2):
                if ib == 2:
                    w0 = alloc_weights_crit(wd[0])
                xkv_t = xkv_pool.tile([P, 2, E], F32, name="xkv_t", tag="xkv_t")
                nc.sync.dma_start(
                    xkv_t[:],
                    xkv_d[ib * 2 * P:(ib + 1) * 2 * P, :].rearrange(
                        "(i p) c -> p i c", p=P
                    ),
                )
                for sub in range(2):
                    i = 2 * ib + sub
                    hkv_t = ln_tile(
                        xkv_t[:, sub, :], xkv_pool, "hkv_t",
                        apply_eng=nc.gpsimd if i % 2 else nc.vector,
                    )
                    for e in range(EC):
                        transpose_block(
                            hkvT[e][:, i * P:(i + 1) * P],
                            hkv_t[:, e * P:(e + 1) * P],
                            "scalar" if (i + e) % 2 else "vector",
                        )

        w0 = alloc_weights_rest(w0, wd[0])

        # Work pools.
        work = ctx.enter_context(tc.tile_pool(name="work", bufs=1))
        big = ctx.enter_context(tc.tile_pool(name="big", bufs=1))
        ex_pool = ctx.enter_context(tc.tile_pool(name="ex", bufs=5))

        for l in range(L):
            if l == 0:
                wt = w0
            else:
                wt = alloc_weights_crit(wd[l])
                wt = alloc_weights_rest(wt, wd[l])
            wq_sb, wk_sb, wv_sb, wo_sb = wt["wq"], wt["wk"], wt["wv"], wt["wo"]
            w1_sb, w2_sb = wt["w1"], wt["w2"]
            bq_sb, bk_sb, b1_sb, bo_sb, b2_sb = (
                wt["bq"], wt["bk"], wt["b1"], wt["bo"], wt["b2"]
            )

            # ---- LN1(x_q) and transpose -> hqT [E, NQ] ----
            hqT = [
                work.tile([P, NQ], BF16, name=f"hqT{e}", tag=f"actT{e}")
                for e in range(EC)
            ]
            for qc in range(QC):
                hq_t = ln_tile(xq[qc], work, "hq_t")
                for e in range(EC):
                    transpose_block(
                        hqT[e][:, qc * P:(qc + 1) * P],
                        hq_t[:, e * P:(e + 1) * P],
                        "scalar" if (qc + e) % 2 else "vector",
                    )

            # ---- q^T = wq^T @ hq^T + bq  [E, NQ] ----
            qT = [
                work.tile([P, NQ], BF16, name=f"qT{m}", tag=f"qT{m}")
                for m in range(EC)
            ]
            for m in range(EC):
                ps = pp_pool.tile([P, E], F32, name="pp", tag="pp")
                for kk in range(EC):
                    nc.tensor.matmul(
                        ps[:],
                        wq_sb[:, kk * E + m * P: kk * E + (m + 1) * P],
                        hqT[kk][:],
                        start=(kk == 0),
                        stop=(kk == EC - 1),
                    )
                nc.vector.tensor_scalar_add(qT[m][:], ps[:], bq_sb[:, m:m + 1])

            # ---- k^T = wk^T @ hkv^T + bk  [E, NKV] ----
            kT = [
                big.tile([P, NKV], BF16, name=f"kT{m}", tag=f"kT{m}", bufs=2)
                for m in range(EC)
            ]
            for m in range(EC):
                for n in range(KN):
                    ps = pp_pool.tile([P, E], F32, name="pp", tag="pp")
                    for kk in range(EC):
                        nc.tensor.matmul(
                            ps[:],
                            wk_sb[:, kk * E + m * P: kk * E + (m + 1) * P],
                            hkvT[kk][:, n * 512:(n + 1) * 512],
                            start=(kk == 0),
                            stop=(kk == EC - 1),
                        )
                    nc.vector.tensor_scalar_add(
                        kT[m][:, n * 512:(n + 1) * 512], ps[:], bk_sb[:, m:m + 1]
                    )

            # ---- v = hkv @ wv  [NKV, E], stored per key-chunk with a ones
            #      column per head: v_aug[m] is [128, H, DH+1] ----
            v_aug = [
                big.tile([P, H * (DH + 1)], BF16, name=f"vaug{m}", tag=f"vaug{m}", bufs=2)
                for m in range(KC)
            ]
            for m in range(KC):
                nc.vector.memset(
                    v_aug[m][:].rearrange("p (h d) -> p h d", h=H)[:, :, DH:DH + 1],
                    1.0,
                )
                ps = pp_pool.tile([P, E], F32, name="pp", tag="pp")
                for kk in range(EC):
                    nc.tensor.matmul(
                        ps[:],
                        hkvT[kk][:, m * P:(m + 1) * P],
                        wv_sb[:, kk * E:(kk + 1) * E],
                        start=(kk == 0),
                        stop=(kk == EC - 1),
                    )
                va = v_aug[m][:].rearrange("p (h d) -> p h d", h=H)
                nc.vector.tensor_copy(
                    va[:, :, 0:DH],
                    ps[:].rearrange("p (h d) -> p h d", h=H),
                )

            # ---- attention, head by head; writes attnout^T directly ----
            aoT = [
                work.tile([P, NQ], BF16, name=f"aoT{e}", tag=f"aoT{e}")
                for e in range(EC)
            ]
            for h in range(H):
                fh, r0 = h // 2, (h % 2) * DH
                # out^T accumulator: rows 0..63 = unnormalized attn@v for this
                # head (transposed), row 64 = softmax denominator per query.
                ps_oT = att_pool.tile([P, E], F32, name="ps_oT", tag="att")
                for g in range(KC // 2):
                    ps_s = ss_pool.tile([P, 2, NQ], F32, name="ps_s", tag="ss")
                    for sub in range(2):
                        m = 2 * g + sub
                        nc.tensor.matmul(
                            ps_s[:, sub, :],
                            kT[fh][r0:r0 + DH, m * P:(m + 1) * P],
                            qT[fh][r0:r0 + DH, :],
                            start=True,
                            stop=True,
                        )
                    ex = ex_pool.tile([P, 2, NQ], BF16, name="ex", tag="ex")
                    nc.scalar.activation(ex[:], ps_s[:], AF.Exp, scale=SCALE)
                    for sub in range(2):
                        m = 2 * g + sub
                        va = v_aug[m][:].rearrange("p (h d) -> p h d", h=H)
                        nc.tensor.matmul(
                            ps_oT[0:DH + 1, :],
                            va[:, h, :],
                            ex[:, sub, :],
                            start=(m == 0),
                            stop=(m == KC - 1),
                        )
                # normalize: aoT rows = unnorm / denom (denom replicated to 64
                # partitions with a K=1 matmul against a ones row).
                rcp = stats_pool.tile([1, NQ], BF16, name="rcp", bufs=2)
                with nc.allow_low_precision(reason="f32r recip row for PE replicate"):
                    nc.vector.reciprocal(rcp[:], ps_oT[DH:DH + 1, :])
                ps_rep = pp_pool.tile([P, E], F32, name="pp", tag="pp")
                nc.tensor.matmul(
                    ps_rep[0:DH, :],
                    ones_row[:],
                    rcp[:],
                    start=True,
                    stop=True,
                )
                u_sb = work.tile([P, NQ], BF16, name="u_sb", bufs=2)
                if h % 2:
                    nc.scalar.copy(u_sb[0:DH, :], ps_oT[0:DH, :])
                else:
                    nc.vector.tensor_copy(u_sb[0:DH, :], ps_oT[0:DH, :])
                nc.vector.tensor_mul(
                    aoT[fh][r0:r0 + DH, :], u_sb[0:DH, :], ps_rep[0:DH, :]
                )

            # ---- out-proj, residual ----
            for qc in range(QC):
                ps = pp_pool.tile([P, E], F32, name="pp", tag="pp")
                for kk in range(EC):
                    nc.tensor.matmul(
                        ps[:],
                        aoT[kk][:, qc * P:(qc + 1) * P],
                        wo_sb[:, kk * E:(kk + 1) * E],
                        start=(kk == 0),
                        stop=(kk == EC - 1),
                    )
                nc.vector.tensor_add(ps[:], ps[:], bo_sb[:])
                nc.vector.tensor_add(xq[qc], xq[qc], ps[:])

            # ---- LN2 + transpose -> h2T ----
            h2T = [
                work.tile([P, NQ], BF16, name=f"h2T{e}", tag=f"actT{e}")
                for e in range(EC)
            ]
            for qc in range(QC):
                h2_t = ln_tile(xq[qc], work, "hq_t")
                for e in range(EC):
                    transpose_block(
                        h2T[e][:, qc * P:(qc + 1) * P],
                        h2_t[:, e * P:(e + 1) * P],
                        "scalar" if (qc + e) % 2 else "vector",
                    )

            # ---- FFN1: g^T = gelu(w1^T @ h2^T + b1)  [MLP, NQ] ----
            gT = [
                big.tile([P, NQ], BF16, name=f"gT{m}", tag=f"gT{m}")
                for m in range(MC)
            ]
            for m in range(MC):
                ps = pp_pool.tile([P, E], F32, name="pp", tag="pp")
                for kk in range(EC):
                    nc.tensor.matmul(
                        ps[:],
                        w1_sb[:, kk * MLP + m * P: kk * MLP + (m + 1) * P],
                        h2T[kk][:],
                        start=(kk == 0),
                        stop=(kk == EC - 1),
                    )
                nc.scalar.activation(gT[m][:], ps[:], AF.Gelu, bias=b1_sb[:, m:m + 1])

            # ---- FFN2 + residual ----
            for qc in range(QC):
                ps = pp_pool.tile([P, E], F32, name="pp", tag="pp")
                for m in range(MC):
                    nc.tensor.matmul(
                        ps[:],
                        gT[m][:, qc * P:(qc + 1) * P],
                        w2_sb[:, m * E:(m + 1) * E],
                        start=(m == 0),
                        stop=(m == MC - 1),
                    )
                nc.vector.tensor_add(ps[:], ps[:], b2_sb[:])
                nc.vector.tensor_add(xq[qc], xq[qc], ps[:])

        for qc in range(QC):
            nc.sync.dma_start(y_d[qc * P:(qc + 1) * P, :], xq[qc])

    nc.compile()
    return nc


def get_nc():
    if "nc" not in _CACHE:
        _CACHE["nc"] = _build()
    return _CACHE["nc"]


def _rearr(w, k):
    """[k*128, C] row-major -> [128, k*C] with free layout (chunk, col)."""
    c = w.shape[1]
    return np.ascontiguousarray(
        w.reshape(k, P, c).transpose(1, 0, 2).reshape(P, k * c)
    )


def _cols(v):
    """[k*128] -> [128, k]: column m holds v[m*128:(m+1)*128]."""
    k = v.shape[0] // P
    return np.ascontiguousarray(v.reshape(k, P).T)


def _bf16(a):
    return np.asarray(a, dtype=np.float32).astype(ml_dtypes.bfloat16)


def kernel(**inputs) -> np.ndarray:
    x_q = np.asarray(inputs["x_q"], np.float32)
    x_kv = np.asarray(inputs["x_kv"], np.float32)
    wq = np.asarray(inputs["wq"], np.float32)
    wkv = np.asarray(inputs["wkv"], np.float32)
    wo = np.asarray(inputs["wo"], np.float32)
    bo = np.asarray(inputs["bo"], np.float32)
    w1 = np.asarray(inputs["w1"], np.float32)
    b1 = np.asarray(inputs["b1"], np.float32)
    w2 = np.asarray(inputs["w2"], np.float32)
    b2 = np.asarray(inputs["b2"], np.float32)
    ln1_g = np.asarray(inputs["ln1_g"], np.float32)
    ln1_b = np.asarray(inputs["ln1_b"], np.float32)
    ln2_g = np.asarray(inputs["ln2_g"], np.float32)
    ln2_b = np.asarray(inputs["ln2_b"], np.float32)

    # Host-side folding of LN affine params into the projection weights.
    shared = {}
    for l in range(L):
        wk_f = wkv[l][:, :E]
        wv_f = wkv[l][:, E:]
        wq_eff = ln1_g[l][:, None] * wq[l]
        wk_eff = ln1_g[l][:, None] * wk_f
        wv_eff = ln1_g[l][:, None] * wv_f
        bq_eff = ln1_b[l] @ wq[l]
        bk_eff = ln1_b[l] @ wk_f
        bv_eff = ln1_b[l] @ wv_f
        bo_eff = bo[l] + bv_eff @ wo[l]
        w1_eff = ln2_g[l][:, None] * w1[l]
        b1_eff = ln2_b[l] @ w1[l] + b1[l]
        shared.update({
            f"wq{l}": _rearr(_bf16(wq_eff), EC),
            f"wk{l}": _rearr(_bf16(wk_eff), EC),
            f"wv{l}": _rearr(_bf16(wv_eff), EC),
            f"wo{l}": _rearr(_bf16(wo[l]), EC),
            f"w1{l}": _rearr(_bf16(w1_eff), EC),
            f"w2{l}": _rearr(_bf16(w2[l]), MC),
            f"bq{l}": _cols(bq_eff),
            f"bk{l}": _cols(bk_eff),
            f"b1{l}": _cols(b1_eff),
            f"bo{l}": np.ascontiguousarray(np.broadcast_to(bo_eff, (P, E))),
            f"b2{l}": np.ascontiguousarray(np.broadcast_to(b2[l], (P, E))),
        })

    in_maps = []
    for c in range(8):
        b, qc = c // 4, c % 4
        m = dict(shared)
        m["xq"] = np.ascontiguousarray(x_q[b, qc * NQ:(qc + 1) * NQ, :])
        m["xkv"] = np.ascontiguousarray(x_kv[b])
        in_maps.append(m)

    nc = get_nc()
    res = bass_utils.run_bass_kernel_spmd(nc, in_maps, core_ids=list(range(8)))

    out = np.empty((2, 2048, E), np.float32)
    for c in range(8):
        b, qc = c // 4, c % 4
        out[b, qc * NQ:(qc + 1) * NQ, :] = res.results[c]["y"]
    return out



# revision 7
# speedup vs baseline: 1.3181x; 1.3181x over previous
"""Trainium2 Bass kernel for a 2-layer cross-encoder (CrossEncoder).

Model: B=2, NQ=NKV=2048, E=512, H=8 (d_head=64), MLP=2048, depth=2, fp32 I/O.

Sharding (8 cores, no collectives): core c handles batch b=c//4 and query
rows [qc*512, (qc+1)*512) with qc=c%4.  Each core computes the full KV
projections for its batch so every core produces its output slice
independently.

Speed strategy (vs bf16 baseline):
- All big matmuls run fp8e4 with perf_mode=DoubleRow (two 128-deep K planes
  per instruction at 0.5 cycles/row).  Weights are pre-scaled by WS=64 on
  the host so w*64 sits in fp8e4's normal range; the 1/64 is folded into
  the PSUM-evacuation ops (or exp/gelu scale args) for free.
- Scores contract over only d_head=64, too shallow for a real DoubleRow
  pair.  Each per-head qT operand is stored as [128, 2, NQ] with the head's
  64 rows alive in plane 0 and ZEROS elsewhere; the kT window supplies a
  garbage second plane which is annihilated by the zero rhs plane.  This
  halves score matmul time (256 cycles per 128-key chunk).
- Softmax denominator comes free from a ones-column appended per head to V
  (row 64 of the attn@V PSUM accumulator).
- k-projection bias is dropped entirely: softmax over keys is invariant to
  the per-query constant q.bk.
- bo / b2 biases enter via an extra contraction plane in wo / w2 (constant
  ones plane in aoT / gT), so residual evac is a single scalar_tensor_tensor.
- exp / gelu / LN-sqrt are the only Activation-engine work (exp dominates);
  every copy/evac is spread across DVE and GpSimd; layer-1 K/V projection
  is interleaved under layer-0's attention.
"""

import numpy as np
import ml_dtypes

import concourse.bass as bass
import concourse.bacc as bacc
import concourse.mybir as mybir
import concourse.tile as tile
from concourse import bass_utils, masks
from contextlib import ExitStack

P = 128
E = 512
EC = E // P        # 4 chunks of the embedding dim
NQ = 512           # query rows per core
QC = NQ // P       # 4 query chunks
NKV = 2048
KC = NKV // P      # 16 key chunks of 128
KN = NKV // 512    # 4 key windows of 512
H = 8
DH = 64
MLP = 2048
MC = MLP // P      # 16 mlp chunks of 128
L = 2
LN_EPS = 1e-5
F32 = mybir.dt.float32
BF16 = mybir.dt.bfloat16
FP8 = mybir.dt.float8e4
AF = mybir.ActivationFunctionType
ALU = mybir.AluOpType
DR = mybir.MatmulPerfMode.DoubleRow
VAS = 80           # padded per-head v_aug slot (16B-aligned for dual-fp8 LDW)
SCALE = (E // H) ** -0.5
WS = 64.0          # host-side weight scale (fp8 range centering)
AOS = 32.0         # attn-out scale (keeps aoT out of fp8 subnormals)

_CACHE = {}


def _build():
    """Build the per-core Bass program (identical on all 8 cores)."""
    nc = bacc.Bacc("TRN2", target_bir_lowering=False, debug=False, num_devices=8)

    xq_d = nc.dram_tensor("xq", [NQ, E], F32, kind="ExternalInput").ap()
    xkv_d = nc.dram_tensor("xkv", [NKV, E], F32, kind="ExternalInput").ap()
    wd = []
    for l in range(L):
        wd.append({
            "wq": nc.dram_tensor(f"wq{l}", [P, EC, E], FP8, kind="ExternalInput").ap(),
            "wk": nc.dram_tensor(f"wk{l}", [P, EC, E], FP8, kind="ExternalInput").ap(),
            "wv": nc.dram_tensor(f"wv{l}", [P, EC, E], FP8, kind="ExternalInput").ap(),
            "wo": nc.dram_tensor(f"wo{l}", [P, EC + 2, E], FP8, kind="ExternalInput").ap(),
            "w1": nc.dram_tensor(f"w1{l}", [P, EC, MLP], FP8, kind="ExternalInput").ap(),
            "w2": nc.dram_tensor(f"w2{l}", [P, MC + 2, E], FP8, kind="ExternalInput").ap(),
            "bq": nc.dram_tensor(f"bq{l}", [P, EC], F32, kind="ExternalInput").ap(),
            "b1": nc.dram_tensor(f"b1{l}", [P, MC], F32, kind="ExternalInput").ap(),
        })
    y_d = nc.dram_tensor("y", [NQ, E], F32, kind="ExternalOutput").ap()

    with tile.TileContext(nc) as tc, ExitStack() as ctx:
        const_pool = ctx.enter_context(tc.tile_pool(name="const", bufs=1))
        ident = const_pool.tile([P, P], BF16)
        masks.make_identity(nc, ident)
        ones_row = const_pool.tile([1, DH], BF16)
        nc.gpsimd.memset(ones_row[:], 1.0)
        eps_col = const_pool.tile([P, 1], F32)
        nc.gpsimd.memset(eps_col[:], LN_EPS)

        stats_pool = ctx.enter_context(tc.tile_pool(name="stats", bufs=12))

        # ---- persistent activation storage ----
        xq_pool = ctx.enter_context(tc.tile_pool(name="xq", bufs=1))
        hkvT_pool = ctx.enter_context(tc.tile_pool(name="hkvT", bufs=1))
        kv_pool = ctx.enter_context(tc.tile_pool(name="kv", bufs=2))
        qp_pool = ctx.enter_context(tc.tile_pool(name="qp", bufs=1))
        act_pool = ctx.enter_context(tc.tile_pool(name="act", bufs=2))
        aoT_pool = ctx.enter_context(tc.tile_pool(name="aoT", bufs=1))
        gT_pool = ctx.enter_context(tc.tile_pool(name="gT", bufs=1))
        ex_pool = ctx.enter_context(tc.tile_pool(name="ex", bufs=4))
        w_pool = ctx.enter_context(tc.tile_pool(name="w", bufs=2))
        hq_pool = ctx.enter_context(tc.tile_pool(name="hq", bufs=3))

        # PSUM: ss 2x2 banks + oT 2x1 + pp 2x1 = 8 banks.
        ss_pool = ctx.enter_context(tc.tile_pool(name="ss", bufs=2, space="PSUM"))
        oT_pool = ctx.enter_context(tc.tile_pool(name="oTp", bufs=2, space="PSUM"))
        pp_pool = ctx.enter_context(tc.tile_pool(name="pp", bufs=2, space="PSUM"))

        # residual stream
        xq = []
        for i in range(QC):
            t = xq_pool.tile([P, E], F32, name=f"xq{i}", tag=f"xq{i}")
            nc.sync.dma_start(t[:], xq_d[i * P:(i + 1) * P, :])
            xq.append(t[:])

        # hkvT: LN1-core of x_kv transposed, [e-chunk planes, NKV], fp8.
        hkvT = hkvT_pool.tile([P, EC, NKV], FP8, name="hkvT", tag="hkvT")

        # per-head zero-banded qT operands [128, 2, NQ]
        qTp = []
        for h in range(H):
            t = qp_pool.tile([P, 2, NQ], FP8, name=f"qTp{h}", tag=f"qTp{h}")
            nc.gpsimd.memset(t[:], 0.0)
            qTp.append(t)

        # aoT: planes 0-3 = attn-out^T chunks, plane 4 = ones (bias), 5 = zero
        aoT = aoT_pool.tile([P, EC + 2, NQ], FP8, name="aoT", tag="aoT")
        nc.gpsimd.memset(aoT[:, EC, :], 1.0)
        nc.gpsimd.memset(aoT[:, EC + 1, :], 0.0)
        # gT: planes 0-15 = gelu(FFN1)^T chunks, plane 16 = ones, 17 = zero
        gT = gT_pool.tile([P, MC + 2, NQ], FP8, name="gT", tag="gT")
        nc.gpsimd.memset(gT[:, MC, :], 1.0)
        nc.gpsimd.memset(gT[:, MC + 1, :], 0.0)

        def alloc_weights(l):
            w = wd[l]
            d = {}
            for nm, shp in [("wq", [P, EC, E]), ("wk", [P, EC, E]),
                            ("wv", [P, EC, E]), ("wo", [P, EC + 2, E]),
                            ("w1", [P, EC, MLP]), ("w2", [P, MC + 2, E])]:
                d[nm] = w_pool.tile(shp, FP8, name=f"{nm}_sb", tag=nm)
                nc.sync.dma_start(d[nm][:], w[nm])
            d["bq"] = w_pool.tile([P, EC], F32, name="bq_sb", tag="bq")
            nc.sync.dma_start(d["bq"][:], w["bq"])
            d["b1"] = w_pool.tile([P, MC], F32, name="b1_sb", tag="b1")
            nc.sync.dma_start(d["b1"][:], w["b1"])
            return d

        def ln_core(x_t, out_t, apply_eng):
            """out_t (bf16) = (x - mu) * rsqrt(var + eps)."""
            bnst = stats_pool.tile([P, 6], F32, name="bnst", tag="bnst")
            nc.vector.bn_stats(bnst[:], x_t)
            bnag = stats_pool.tile([P, 2], F32, name="bnag", tag="bnag")
            nc.vector.bn_aggr(bnag[:], bnst[:])
            sq = stats_pool.tile([P, 1], F32, name="sq", tag="sq")
            nc.scalar.activation(sq[:], bnag[:, 1:2], AF.Sqrt, bias=eps_col[:])
            rstd = stats_pool.tile([P, 1], F32, name="rstd", tag="rstd")
            nc.vector.reciprocal(rstd[:], sq[:])
            apply_eng.tensor_scalar(
                out_t, x_t, bnag[:, 0:1], rstd[:], op0=ALU.subtract, op1=ALU.mult
            )

        def transpose_pair(srcs, dstT, w0, evac_eng):
            """Transpose two [128, E] bf16 tiles into dstT[:, :, w0:w0+256] (fp8).

            srcs: list of 2 bf16 [P, E] APs (consecutive row chunks).
            dstT: [P, EC, *] fp8 tile; w0: starting column.
            """
            pt = pp_pool.tile([P, NQ], F32, name="pp", tag="pp")
            ptb = pt[:].bitcast(BF16)
            for sub, src in enumerate(srcs):
                for e in range(EC):
                    slot = e * 2 + sub
                    nc.tensor.transpose(
                        ptb[:, slot * P:(slot + 1) * P],
                        src[:, e * P:(e + 1) * P],
                        ident[:],
                    )
            src_v = ptb.rearrange("p (e s c) -> p e s c", e=EC, s=2)
            dst_v = dstT[:, :, w0:w0 + 2 * P].rearrange("p e (s c) -> p e s c", s=2)
            if evac_eng is nc.scalar:
                evac_eng.copy(dst_v, src_v)
            else:
                evac_eng.tensor_copy(dst_v, src_v)

        # ================= prologue: x_kv -> hkvT =================
        w_sb = [None, None]
        with tc.tile_pool(name="xkv", bufs=4) as xkv_pool:
            evac_engs = [nc.vector, nc.scalar, nc.vector, nc.scalar]
            for ib in range(KC // 2):
                if ib == 2:
                    w_sb[0] = alloc_weights(0)
                xkv_t = xkv_pool.tile([P, 2, E], F32, name="xkv_t", tag="xkv_t")
                nc.sync.dma_start(
                    xkv_t[:],
                    xkv_d[ib * 2 * P:(ib + 1) * 2 * P, :].rearrange(
                        "(i p) c -> p i c", p=P
                    ),
                )
                hs = []
                for sub in range(2):
                    h_t = hq_pool.tile([P, E], BF16, name="hkv_t", tag="hkv_t")
                    ln_core(xkv_t[:, sub, :], h_t[:], nc.gpsimd)
                    hs.append(h_t[:])
                transpose_pair(hs, hkvT, ib * 2 * P, evac_engs[ib % 4])
        w_sb[1] = alloc_weights(1)

        def scaled_evac(eng, dst, src):
            """dst = src / WS via DVE tensor_scalar or ACT Copy activation."""
            if eng is nc.scalar:
                eng.activation(dst, src, AF.Copy, scale=1.0 / WS)
            else:
                eng.tensor_scalar_mul(dst, src, 1.0 / WS)

        def emit_kproj_unit(wt, kT_t, fh, n, eng):
            pt = pp_pool.tile([P, NQ], F32, name="pp", tag="pp")
            for j in range(EC // 2):
                nc.tensor.matmul(
                    pt[:],
                    wt["wk"][:, 2 * j:2 * j + 2, fh * P:(fh + 1) * P],
                    hkvT[:, 2 * j:2 * j + 2, n * 512:(n + 1) * 512],
                    start=(j == 0), stop=(j == EC // 2 - 1), perf_mode=DR,
                )
            scaled_evac(
                eng,
                kT_t[fh][:, 4 * n:4 * n + 4, :],
                pt[:].rearrange("p (a b) -> p a b", a=4),
            )

        def emit_vproj_unit(wt, vaug_t, m, eng):
            pt = pp_pool.tile([P, NQ], F32, name="pp", tag="pp")
            for j in range(EC // 2):
                nc.tensor.matmul(
                    pt[:],
                    hkvT[:, 2 * j:2 * j + 2, m * P:(m + 1) * P],
                    wt["wv"][:, 2 * j:2 * j + 2, :],
                    start=(j == 0), stop=(j == EC // 2 - 1), perf_mode=DR,
                )
            scaled_evac(
                eng,
                vaug_t[:, m, :, 0:DH],
                pt[:].rearrange("p (h d) -> p h d", h=H),
            )

        def alloc_kv_tiles():
            kT_t = [
                kv_pool.tile([P, KC + 1, P], FP8, name=f"kT{fh}", tag=f"kT{fh}")
                for fh in range(EC)
            ]
            vaug_t = kv_pool.tile([P, KC, H, VAS], FP8, name="vaug", tag="vaug")
            for fh in range(EC):
                nc.gpsimd.memset(kT_t[fh][:, KC, :], 0.0)
            nc.gpsimd.memset(vaug_t[:, :, :, DH:DH + 1], 1.0)
            return kT_t, vaug_t

        def kv_units(l, kT_t, vaug_t):
            """List of closures: kproj fh0 first, then all vproj, then rest."""
            units = []
            engs = [nc.vector, nc.scalar] if l == 0 else [nc.vector, nc.vector]
            for n in range(KN):
                units.append(lambda n=n, e=engs[n % 2]:
                             emit_kproj_unit(w_sb[l], kT_t, 0, n, e))
            for m in range(KC):
                units.append(lambda m=m, e=engs[m % 2]:
                             emit_vproj_unit(w_sb[l], vaug_t, m, e))
            for fh in range(1, EC):
                for n in range(KN):
                    units.append(lambda fh=fh, n=n, e=engs[(fh + n) % 2]:
                                 emit_kproj_unit(w_sb[l], kT_t, fh, n, e))
            return units

        # layer-0 K/V: emit fully in the prologue.
        kT0, vaug0 = alloc_kv_tiles()
        for u in kv_units(0, kT0, vaug0):
            u()

        kv_tiles = [(kT0, vaug0), (None, None)]

        for l in range(L):
            wt = w_sb[l]
            kT_t, vaug_t = kv_tiles[l]

            # ---- LN1(x_q), transpose -> hqT [e-chunk planes, NQ] fp8 ----
            hqT = act_pool.tile([P, EC, NQ], FP8, name="hqT", tag="actT")
            for g in range(QC // 2):
                hs = []
                for sub in range(2):
                    qc = 2 * g + sub
                    h_t = hq_pool.tile([P, E], BF16, name="hq_t", tag="hq_t")
                    ln_core(xq[qc], h_t[:], nc.gpsimd)
                    hs.append(h_t[:])
                transpose_pair(hs, hqT, g * 2 * P, nc.vector)

            # ---- q^T into zero-banded per-head tiles ----
            for m in range(EC):
                pt = pp_pool.tile([P, NQ], F32, name="pp", tag="pp")
                for j in range(EC // 2):
                    nc.tensor.matmul(
                        pt[:],
                        wt["wq"][:, 2 * j:2 * j + 2, m * P:(m + 1) * P],
                        hqT[:, 2 * j:2 * j + 2, :],
                        start=(j == 0), stop=(j == EC // 2 - 1), perf_mode=DR,
                    )
                nc.vector.tensor_scalar(
                    qTp[2 * m][0:DH, 0, :], pt[0:DH, :],
                    1.0 / WS, wt["bq"][0:DH, m:m + 1],
                    op0=ALU.mult, op1=ALU.add,
                )
                nc.vector.tensor_scalar(
                    qTp[2 * m + 1][DH:P, 0, :], pt[DH:P, :],
                    1.0 / WS, wt["bq"][DH:P, m:m + 1],
                    op0=ALU.mult, op1=ALU.add,
                )

            # ---- attention ----
            if l + 1 < L:
                kv_next = alloc_kv_tiles()
                kv_tiles[l + 1] = kv_next
                inj = kv_units(l + 1, *kv_next)
            else:
                inj = []

            for h in range(H):
                fh, r0 = h // 2, (h % 2) * DH
                ps_oT = oT_pool.tile([DH + 1, NQ], F32, name="ps_oT", tag="oT")
                exs = []

                def attnv(t):
                    nc.tensor.matmul(
                        ps_oT[:],
                        vaug_t[:, 2 * t:2 * t + 2, h, 0:DH + 1],
                        exs[t][:],
                        start=(t == 0), stop=(t == KC // 2 - 1), perf_mode=DR,
                    )

                for t in range(KC // 2):
                    ss = ss_pool.tile([P, 2, NQ], F32, name="ss", tag="ss")
                    nc.tensor.matmul(
                        ss[:, 0, :], kT_t[fh][:, 2 * t:2 * t + 2, :], qTp[h][:],
                        start=True, stop=True, perf_mode=DR,
                    )
                    nc.tensor.matmul(
                        ss[:, 1, :], kT_t[fh][:, 2 * t + 1:2 * t + 3, :], qTp[h][:],
                        start=True, stop=True, perf_mode=DR,
                    )
                    ex = ex_pool.tile([P, 2, NQ], FP8, name="ex", tag="ex")
                    nc.scalar.activation(ex[:], ss[:], AF.Exp, scale=SCALE)
                    exs.append(ex)
                    if t >= 1:
                        attnv(t - 1)
                attnv(KC // 2 - 1)

                # normalize: aoT band = ps_oT[0:64] * AOS * (1/denom replicated)
                rcp = stats_pool.tile([1, NQ], BF16, name="rcp", tag="rcp", bufs=2)
                with nc.allow_low_precision(reason="recip row for replicate"):
                    nc.vector.reciprocal(rcp[:], ps_oT[DH:DH + 1, :])
                rcp_rep = stats_pool.tile(
                    [DH, NQ], BF16, name="rcp_rep", tag="rcp_rep", bufs=2
                )
                nc.gpsimd.partition_broadcast(rcp_rep[:], rcp[:])
                nc.vector.scalar_tensor_tensor(
                    aoT[r0:r0 + DH, fh, :], ps_oT[0:DH, :], AOS, rcp_rep[:],
                    op0=ALU.mult, op1=ALU.mult,
                )

                # inject next layer's K/V projection work under attention
                for u in inj[h * 4:(h + 1) * 4]:
                    u()

            # ---- out-proj (+bo via bias plane), residual ----
            for qc in range(QC):
                pt = pp_pool.tile([P, NQ], F32, name="pp", tag="pp")
                for j in range(EC // 2 + 1):
                    nc.tensor.matmul(
                        pt[:],
                        aoT[:, 2 * j:2 * j + 2, qc * P:(qc + 1) * P],
                        wt["wo"][:, 2 * j:2 * j + 2, :],
                        start=(j == 0), stop=(j == EC // 2), perf_mode=DR,
                    )
                nc.vector.scalar_tensor_tensor(
                    xq[qc], pt[:], 1.0 / (WS * AOS), xq[qc],
                    op0=ALU.mult, op1=ALU.add,
                )

            # ---- LN2 + transpose -> h2T ----
            h2T = act_pool.tile([P, EC, NQ], FP8, name="h2T", tag="actT")
            for g in range(QC // 2):
                hs = []
                for sub in range(2):
                    qc = 2 * g + sub
                    h_t = hq_pool.tile([P, E], BF16, name="hq_t", tag="hq_t")
                    ln_core(xq[qc], h_t[:], nc.gpsimd)
                    hs.append(h_t[:])
                transpose_pair(hs, h2T, g * 2 * P, nc.vector)

            # ---- FFN1: gT = gelu(w1^T @ h2T / WS + b1) ----
            for m in range(MC):
                pt = pp_pool.tile([P, NQ], F32, name="pp", tag="pp")
                for j in range(EC // 2):
                    nc.tensor.matmul(
                        pt[:],
                        wt["w1"][:, 2 * j:2 * j + 2, m * P:(m + 1) * P],
                        h2T[:, 2 * j:2 * j + 2, :],
                        start=(j == 0), stop=(j == EC // 2 - 1), perf_mode=DR,
                    )
                nc.scalar.activation(
                    gT[:, m, :], pt[:], AF.Gelu,
                    bias=wt["b1"][:, m:m + 1], scale=1.0 / WS,
                )

            # ---- FFN2 (+b2 via bias plane) + residual ----
            for qc in range(QC):
                pt = pp_pool.tile([P, NQ], F32, name="pp", tag="pp")
                for j in range(MC // 2 + 1):
                    nc.tensor.matmul(
                        pt[:],
                        gT[:, 2 * j:2 * j + 2, qc * P:(qc + 1) * P],
                        wt["w2"][:, 2 * j:2 * j + 2, :],
                        start=(j == 0), stop=(j == MC // 2), perf_mode=DR,
                    )
                nc.vector.scalar_tensor_tensor(
                    xq[qc], pt[:], 1.0 / WS, xq[qc],
                    op0=ALU.mult, op1=ALU.add,
                )

        for qc in range(QC):
            nc.sync.dma_start(y_d[qc * P:(qc + 1) * P, :], xq[qc])

    nc.compile()
    return nc


def get_nc():
    if "nc" not in _CACHE:
        _CACHE["nc"] = _build()
    return _CACHE["nc"]


def _rearr(w, k):
    """[k*128, C] row-major -> [128, k, C]: plane j holds rows j*128.."""
    c = w.shape[1]
    return np.ascontiguousarray(w.reshape(k, P, c).transpose(1, 0, 2))


def _cols(v):
    """[k*128] -> [128, k]: column m holds v[m*128:(m+1)*128]."""
    k = v.shape[0] // P
    return np.ascontiguousarray(v.reshape(k, P).T)


def _f8(a):
    return np.clip(np.asarray(a, np.float32), -240.0, 240.0).astype(
        ml_dtypes.float8_e4m3
    )


def kernel(**inputs) -> np.ndarray:
    x_q = np.asarray(inputs["x_q"], np.float32)
    x_kv = np.asarray(inputs["x_kv"], np.float32)
    wq = np.asarray(inputs["wq"], np.float32)
    wkv = np.asarray(inputs["wkv"], np.float32)
    wo = np.asarray(inputs["wo"], np.float32)
    bo = np.asarray(inputs["bo"], np.float32)
    w1 = np.asarray(inputs["w1"], np.float32)
    b1 = np.asarray(inputs["b1"], np.float32)
    w2 = np.asarray(inputs["w2"], np.float32)
    b2 = np.asarray(inputs["b2"], np.float32)
    ln1_g = np.asarray(inputs["ln1_g"], np.float32)
    ln1_b = np.asarray(inputs["ln1_b"], np.float32)
    ln2_g = np.asarray(inputs["ln2_g"], np.float32)
    ln2_b = np.asarray(inputs["ln2_b"], np.float32)

    # Host-side folding of LN affine params into the projection weights.
    shared = {}
    for l in range(L):
        wk_f = wkv[l][:, :E]
        wv_f = wkv[l][:, E:]
        wq_eff = ln1_g[l][:, None] * wq[l]
        wk_eff = ln1_g[l][:, None] * wk_f
        wv_eff = ln1_g[l][:, None] * wv_f
        bq_eff = ln1_b[l] @ wq[l]
        # bk is dropped (softmax shift invariance); bv folds into bo.
        bv_eff = ln1_b[l] @ wv_f
        bo_eff = bo[l] + bv_eff @ wo[l]
        w1_eff = ln2_g[l][:, None] * w1[l]
        b1_eff = ln2_b[l] @ w1[l] + b1[l]

        wo_ext = np.zeros((P, EC + 2, E), np.float32)
        wo_ext[:, :EC, :] = _rearr(wo[l] * WS, EC)
        wo_ext[0, EC, :] = bo_eff * WS * AOS
        w2_ext = np.zeros((P, MC + 2, E), np.float32)
        w2_ext[:, :MC, :] = _rearr(w2[l] * WS, MC)
        w2_ext[0, MC, :] = b2[l] * WS

        shared.update({
            f"wq{l}": _f8(_rearr(wq_eff * WS, EC)),
            f"wk{l}": _f8(_rearr(wk_eff * WS, EC)),
            f"wv{l}": _f8(_rearr(wv_eff * WS, EC)),
            f"wo{l}": _f8(wo_ext),
            f"w1{l}": _f8(_rearr(w1_eff * WS, EC)),
            f"w2{l}": _f8(w2_ext),
            f"bq{l}": _cols(bq_eff),
            f"b1{l}": _cols(b1_eff),
        })

    in_maps = []
    for c in range(8):
        b, qc = c // 4, c % 4
        m = dict(shared)
        m["xq"] = np.ascontiguousarray(x_q[b, qc * NQ:(qc + 1) * NQ, :])
        m["xkv"] = np.ascontiguousarray(x_kv[b])
        in_maps.append(m)

    nc = get_nc()
    res = bass_utils.run_bass_kernel_spmd(nc, in_maps, core_ids=list(range(8)))

    out = np.empty((2, 2048, E), np.float32)
    for c in range(8):
        b, qc = c // 4, c % 4
        out[b, qc * NQ:(qc + 1) * NQ, :] = res.results[c]["y"]
    return out
